# revision 11
# baseline (speedup 1.0000x reference)
"""Causal single-head attention block for Trainium2, SPMD across 8 NeuronCores.

Problem (hardcoded):
    x:     [4, 2048, 1024] f32
    w_qkv: [1024, 3072]    f32   (q | k | v column blocks)
    w_out: [1024, 1024]    f32
    b_out: [1024]          f32
    y = softmax(causal(q @ k.T / 32)) @ v @ w_out + b_out     -> [4, 2048, 1024]

Algebraic folding (host-side, fp32):
    sim  = (x wq)(x wk)^T = x (wq wk^T) x^T          -> Mq  = wq @ wk.T
    out  = attn (x wv) wo = attn x (wv wo)           -> Wvo = wv @ w_out
so the device kernel never materializes Q/K/V: it computes
    QM^T = Mq^T x_q^T   (local queries only)
    sim  = QM x^T       (x^T SBUF-resident)
    attnX = softmax(causal(sim)) @ x                 (x rows SBUF-resident)
    y    = attnX @ Wvo + b

Sharding: 2 cores per batch element. Within a batch, the 16 query subtiles of
128 rows are dealt round-robin to the core pair (core parity h gets subtiles
s = 2k + h, k = 0..7) so both cores see the identical causal work profile
(512-key chunk counts [1,1,2,2,3,3,4,4]) and a single SPMD program serves all
8 cores; per-core behavior differs only through input data.

Schedule notes (v4, driven by ntff traces of earlier revisions):
  * The PE runs 512-free bf16 matmuls at a 216ns cadence once warm; wins are
    at the edges. HAM grants ~133.1us of full-rate PE per core then clamps
    to half rate, and any PE idle >~2us costs an extra ~3.4us half-rate
    quantum on resume, so the schedule avoids both long stalls and overrun.
  * Phase 0 accumulates all 8 QM^T d-chunks in the 8 PSUM banks at once
    (consumption ~1.7us per 512KB feed step vs ~1.4us arrival), with an
    8-matmul warm-up bridging the trigger preamble to first-chunk arrival.
    The PSUM->SBUF casts alternate vector/scalar, and the first sim
    subtile's contraction is split i=0..3 / i=4..7 so its first half only
    waits for half the casts.
  * DMA trigger instructions cost ~0.6-0.9us on the issuing engine and the
    DGE queue blocks the engine when its semaphore rotation (~4 deep) is
    exhausted, so: sync owns the phase-0-critical stream plus the early
    tail, scalar owns a 14-trigger static prefix and issues the rest in
    small stages between compute phases (its engine is otherwise idle
    there), and all 16 x row tiles are pinned in SBUF (one read each)
    instead of re-streaming rows per PV group (saves 3MB of wire).
  * PV skips the two fully-masked 128-key blocks of each even subtile
    (128-wide matmuls on the odd-subtile half only): -3.4us of PE.
  * proj groups 2/3 draw their y PSUM from the then-idle accp pool so the
    tensor engine never waits on a PSUM WAR against the immediately
    preceding PV accumulation.

All matmul operands are bf16 (PSUM accumulation in fp32; softmax statistics
in fp32): the elementwise rounding step is 4x fp32r's, far inside the
tolerance, and bf16 enables fast weight load + halves DMA/DVE traffic.
"""

import numpy as np

import concourse.mybir as mybir
import concourse.tile as tile
from concourse import bacc
from concourse.bass_utils import run_bass_kernel_spmd

FP32 = mybir.dt.float32
BF16 = mybir.dt.bfloat16
AF = mybir.ActivationFunctionType
ALU = mybir.AluOpType

B, S, D, NI, NO = 4, 2048, 1024, 1024, 1024
NCORES = 8
P = 128
DC = D // P    # 8 contraction chunks for the projections
IC = NI // P   # 8 inner-dim chunks
NSUB = 8       # local 128-row query subtiles per core
CC = [k // 2 + 1 for k in range(NSUB)]  # 512-key chunks per local subtile
SCALE = float(NI) ** -0.5
NEG = -1.0e9
NWARM = 8

_CACHED = {}


def _build():
    nc = bacc.Bacc(None, target_bir_lowering=False, debug=False, num_devices=NCORES)

    xT = nc.dram_tensor("xT", [D, S], BF16, kind="ExternalInput").ap()
    mq_d = nc.dram_tensor("mq", [D, D], BF16, kind="ExternalInput").ap()
    # local query columns of x^T, packed [low 512 | high 512]
    xq_d = nc.dram_tensor("xq", [D, NSUB * P], BF16, kind="ExternalInput").ap()
    xR = nc.dram_tensor("xR", [S, D], BF16, kind="ExternalInput").ap()
    wvo_d = nc.dram_tensor("wvo", [NI, NO], BF16, kind="ExternalInput").ap()
    masks = nc.dram_tensor("masks", [NSUB, P, 512], BF16, kind="ExternalInput").ap()
    bb = nc.dram_tensor("bb", [P, NO], FP32, kind="ExternalInput").ap()
    y = nc.dram_tensor("y", [NSUB * P, NO], BF16, kind="ExternalOutput").ap()

    with tile.TileContext(nc) as tc:
        with (
            tc.tile_pool(name="const", bufs=1) as constp,
            tc.tile_pool(name="xtpool", bufs=2 * IC) as xtp,
            tc.tile_pool(name="qtpool0", bufs=IC) as qtp0,
            tc.tile_pool(name="qtpool1", bufs=IC) as qtp1,
            tc.tile_pool(name="wpool", bufs=DC) as wp,
            tc.tile_pool(name="xqpool", bufs=2 * DC) as xqp,
            tc.tile_pool(name="vfixp", bufs=16) as vfixp,
            tc.tile_pool(name="wopool", bufs=DC) as wop,
        ):
            # two separate tile arrays per key-half: a write to the high half
            # must not create a false whole-tile WAR hazard against sim reads
            # of the low half (tile dependencies are tracked per-tile)
            XTa = [xtp.tile([P, S // 2], BF16, name=f"xta{i}", tag="xt")
                   for i in range(IC)]
            XTb = [xtp.tile([P, S // 2], BF16, name=f"xtb{i}", tag="xt")
                   for i in range(IC)]
            # per-qh-half QM^T tiles: keeps group 0's sim free of any
            # dependency on the second half's PSUM->SBUF copies
            QMT = [
                [qtp0.tile([P, 512], BF16, name=f"qt0_{i}", tag="qt0")
                 for i in range(IC)],
                [qtp1.tile([P, 512], BF16, name=f"qt1_{i}", tag="qt1")
                 for i in range(IC)],
            ]
            mq = [wp.tile([P, D], BF16, name=f"mq{d}", tag="w")
                  for d in range(DC)]
            xqlo = [xqp.tile([P, 512], BF16, name=f"xql{d}", tag="xq")
                    for d in range(DC)]
            xqhi = [xqp.tile([P, 512], BF16, name=f"xqh{d}", tag="xq")
                    for d in range(DC)]
            mask_sb = constp.tile([P, NSUB, 512], BF16, name="mask_sb", tag="mask")
            vfix = [vfixp.tile([P, NI], BF16, name=f"vfix{t}", tag="vfix")
                    for t in range(16)]
            wo = [wop.tile([P, NO], BF16, name=f"wo{d}", tag="wo")
                  for d in range(DC)]
            b_sb = constp.tile([P, NO], FP32, name="b_sb", tag="b")

            def ld_xq(d, half, eng):
                t = xqlo[d] if half == 0 else xqhi[d]
                eng.dma_start(out=t[:],
                              in_=xq_d[P * d:P * (d + 1),
                                       512 * half:512 * (half + 1)])

            def ld_mask(k, eng):
                eng.dma_start(out=mask_sb[:, k, :], in_=masks[k])

            def ld_vfx(t, eng):
                eng.dma_start(out=vfix[t][:], in_=xR[P * t:P * (t + 1), :])

            def ld_wo(d, eng):
                eng.dma_start(out=wo[d][:], in_=wvo_d[P * d:P * (d + 1), :])

            # ---- phase-0-critical feed: mq + xq-low interleaved so step d's
            # pair lands on opposite queues simultaneously
            for d in range(DC):
                if d % 2 == 0:
                    nc.sync.dma_start(out=mq[d][:], in_=mq_d[P * d:P * (d + 1), :])
                    ld_xq(d, 0, nc.scalar)
                else:
                    nc.scalar.dma_start(out=mq[d][:], in_=mq_d[P * d:P * (d + 1), :])
                    ld_xq(d, 0, nc.sync)
            for i in range(IC):
                eng = nc.sync if i % 2 == 0 else nc.scalar
                eng.dma_start(out=XTa[i][:], in_=xT[P * i:P * (i + 1), 0:1024])
            ld_mask(0, nc.sync)
            ld_mask(1, nc.scalar)
            ld_mask(2, nc.sync)
            ld_mask(3, nc.scalar)
            # scalar's static prefix stops at 14 triggers so its engine is
            # free for the phase-0 casts by ~24us; the sync tail continues in
            # need order (sim-group-2 keys, qmt1's xq-high, masks 4/6, first
            # PV x rows). The tail is gated with tile_wait_until so the
            # scheduler cannot hoist it in front of the phase-0 stream (it
            # schedules by dependency, not program order).
            with tc.tile_wait_until(0.016):
                for i in range(0, IC, 2):
                    nc.sync.dma_start(out=XTb[i][:],
                                      in_=xT[P * i:P * (i + 1), 1024:2048])
                for d in range(0, DC, 2):
                    ld_xq(d, 1, nc.sync)
                ld_mask(4, nc.sync)
                ld_mask(6, nc.sync)
                ld_vfx(0, nc.sync)
                ld_vfx(2, nc.sync)

            # ---- Phase 0: QM^T = Mq^T @ xQ for queries 0:512 (groups 0/1),
            # all 8 output chunks accumulating in the 8 PSUM banks at once so
            # the PE tracks the dual-queue feed with no round structure.
            with tc.tile_pool(name="qacc", bufs=8, space="PSUM") as qacc:
                # HAM warm-up: bridges the ~3us between the trigger preamble
                # and the first mq/xq chunk landing, and starts the clock
                # ramp-up early. Sized to end roughly at first-chunk arrival.
                warm = constp.tile([P, 512], BF16, name="warm", tag="warm")
                nc.vector.memset(warm[:], 0.0)
                wps = qacc.tile([P, 512], FP32, name="warmps", tag="qa")
                for w in range(NWARM):
                    nc.tensor.matmul(
                        wps[:], warm[:, 0:P], warm[:],
                        start=(w == 0), stop=(w == NWARM - 1),
                    )
                pss = [
                    qacc.tile([P, 512], FP32, name=f"ps_qt0_{j}", tag="qa")
                    for j in range(IC)
                ]
                for d in range(DC):
                    for j in range(IC):
                        nc.tensor.matmul(
                            pss[j][:],
                            mq[d][:, P * j:P * (j + 1)],
                            xqlo[d][:],
                            start=(d == 0), stop=(d == DC - 1),
                        )
                # casts alternate vector/scalar; bank j's cast chains off its
                # own d=7 matmul (banks complete 216ns apart), so each
                # engine's 4-cast chain ends ~1.7us after the last matmul
                for j in range(IC):
                    if j % 2 == 0:
                        nc.vector.tensor_copy(QMT[0][j][:], pss[j][:])
                    else:
                        nc.scalar.activation(QMT[0][j][:], pss[j][:], AF.Copy)

            # scalar stage: qmt1's xq-high + late masks, gated past the
            # phase-0 casts (~25us) so the trigger instructions can't block
            # the scalar engine before them
            with tc.tile_wait_until(0.027):
                for d in range(1, DC, 2):
                    ld_xq(d, 1, nc.scalar)
                ld_mask(5, nc.scalar)
                ld_mask(7, nc.scalar)

            with (
                tc.tile_pool(name="accp", bufs=4, space="PSUM") as accp,
                tc.tile_pool(name="opp", bufs=4, space="PSUM") as opp,
                tc.tile_pool(name="ppool", bufs=4) as ppool,
                tc.tile_pool(name="ptpool", bufs=2) as ptpool,
                tc.tile_pool(name="otpool", bufs=2 * IC) as otpool,
                tc.tile_pool(name="ypool", bufs=4) as ypool,
                tc.tile_pool(name="stp", bufs=24) as stp,
            ):
                PS = {}
                PTB = {}
                OT = {}

                def qmt1_phase():
                    # second query half (groups 2/3): d-inner accumulation
                    # through the accp ping-pong, overlapped with the
                    # group-0/1 attention pipeline
                    for i in range(IC):
                        ps = accp.tile([P, 512], FP32, name="ps_qt1",
                                       tag="acc")
                        for d in range(DC):
                            nc.tensor.matmul(
                                ps[:], mq[d][:, P * i:P * (i + 1)],
                                xqhi[d][:],
                                start=(d == 0), stop=(d == DC - 1),
                            )
                        if i % 2 == 0:
                            nc.vector.tensor_copy(QMT[1][i][:], ps[:])
                        else:
                            nc.scalar.activation(QMT[1][i][:], ps[:], AF.Copy)

                def sim_phase(g):
                    L = g + 1
                    nt = 4 * L
                    ptb = ptpool.tile([P, 16, 256], BF16, name=f"ptb{g}",
                                      tag="ptb")
                    for k in (2 * g, 2 * g + 1):
                        # exact causal need: subtile k covers 256*(k+1)
                        # keys. Even k: k/2 full 512 chunks + a 256-wide
                        # diagonal chunk; odd k: (k+1)/2 chunks with the
                        # diagonal mask in the last 512 chunk.
                        even = (k % 2 == 0)
                        ndiag = 256 if even else 512
                        doff = 256 * k if even else 512 * (L - 1)
                        nfull = k // 2 if even else L - 1
                        p_t = ppool.tile([P, 4 * 512], BF16, name=f"p{k}",
                                         tag="p")
                        sums = stp.tile([P, 4], FP32, name=f"sums{k}",
                                        tag="sums")
                        # diagonal chunk first: its mask+exp chain overlaps
                        # the remaining chunks' matmuls
                        for ci, kc in enumerate([-1] + list(range(nfull))):
                            ps = accp.tile([P, 512], FP32, name="ps_sim",
                                           tag="acc")
                            if kc < 0:
                                off, w = doff, ndiag
                            else:
                                off, w = 512 * kc, 512
                            psv = ps[:, 0:w]
                            # group 0's first chunks split the contraction
                            # in half so the first segment only waits on the
                            # first half of the phase-0 casts
                            segs = ((0, 4), (4, 8)) if g == 0 else ((0, 8),)
                            for i0, i1 in segs:
                                for i in range(i0, i1):
                                    xth = XTa[i] if off < 1024 else XTb[i]
                                    nc.tensor.matmul(
                                        psv,
                                        QMT[k // 4][i][:, P * (k % 4):
                                                       P * (k % 4) + P],
                                        xth[:, off % 1024:off % 1024 + w],
                                        start=(i == 0), stop=(i == IC - 1),
                                    )
                            if kc < 0:
                                nc.vector.tensor_tensor(
                                    out=psv, in0=psv,
                                    in1=mask_sb[:, k, 0:w],
                                    op=ALU.add,
                                )
                            nc.scalar.activation(
                                p_t[:, off:off + w], psv, AF.Exp,
                                scale=SCALE, accum_out=sums[:, ci:ci + 1],
                            )
                        ssum = stp.tile([P, 1], FP32, name=f"ssum{k}", tag="ss")
                        nc.vector.tensor_reduce(
                            ssum[:], sums[:, :nfull + 1],
                            axis=mybir.AxisListType.X,
                            op=ALU.add,
                        )
                        rsum = stp.tile([P, 1], FP32, name=f"rsum{k}", tag="rs")
                        nc.vector.reciprocal(rsum[:], ssum[:])
                        nc.vector.tensor_scalar_mul(
                            p_t[:, :256 * (k + 1)], p_t[:, :256 * (k + 1)],
                            rsum[:]
                        )
                        PS[k] = p_t
                        # one XBAR DMA per subtile transposes the whole
                        # normalized-probability slab into blocked
                        # [key, t, q] layout; issued here so it overlaps
                        # the previous group's PV/projection work. The
                        # even subtile's last two key blocks are fully
                        # masked and simply never written nor read.
                        half = slice(0, P) if even else slice(P, 256)
                        nc.sync.dma_start(
                            out=ptb[:, 0:2 * (k + 1), half],
                            in_=p_t[:, 0:256 * (k + 1)],
                            transpose=True,
                        )
                    PTB[g] = ptb

                def pv_phase(g):
                    L = g + 1
                    ops = [
                        opp.tile([P, 512], FP32, name=f"op{g}_{j}", tag="op")
                        for j in range(4)
                    ]
                    nt = 4 * L
                    ptb = PTB[g]
                    for t in range(nt):
                        v_t = vfix[t]
                        # the even subtile's last two key blocks are fully
                        # masked: matmul only the odd-subtile query half
                        narrow = t >= nt - 2
                        for m in range(IC):
                            if narrow:
                                out = ops[m // 2][:, 256 * (m % 2) + P:
                                                  256 * (m % 2) + 256]
                                rhs = ptb[:, t, P:256]
                            else:
                                out = ops[m // 2][:, 256 * (m % 2):
                                                  256 * (m % 2) + 256]
                                rhs = ptb[:, t, :]
                            # one accumulation group per PSUM bank: start
                            # only on the bank's first matmul (whole-bank
                            # pending-zero makes the sibling column-half's
                            # first write an overwrite), stop on its last
                            nc.tensor.matmul(
                                out,
                                v_t[:, P * m:P * (m + 1)],
                                rhs,
                                start=(t == 0 and m % 2 == 0),
                                stop=(t == nt - 1 and m % 2 == 1),
                            )

                    oT = []
                    for m in range(IC):
                        ot = otpool.tile([P, 256], BF16, name=f"ot{g}_{m}",
                                         tag="ot")
                        nc.vector.tensor_copy(
                            ot[:],
                            ops[m // 2][:, 256 * (m % 2):256 * (m % 2) + 256]
                        )
                        oT.append(ot)
                    OT[g] = oT

                def proj_phase(g):
                    # deferred past the next group's sim matmuls so the
                    # serialized attnX^T PSUM->SBUF casts don't stall the
                    # tensor engine between PV and projection
                    k0, k1 = 2 * g, 2 * g + 1
                    oT = OT[g]
                    # groups 0/1 draw y PSUM from opp (accp is busy with the
                    # interleaved sim); groups 2/3 draw from the then-idle
                    # accp so the first y matmul never waits a WAR against
                    # the PV accumulation that just stopped
                    ypool_ps = opp if g < 2 else accp
                    ytag = "op" if g < 2 else "acc"
                    for col, k in enumerate((k0, k1)):
                        for oh in range(2):
                            ps = ypool_ps.tile([P, 512], FP32, name="ps_y",
                                               tag=ytag)
                            for i in range(IC):
                                nc.tensor.matmul(
                                    ps[:],
                                    oT[i][:, P * col:P * (col + 1)],
                                    wo[i][:, 512 * oh:512 * (oh + 1)],
                                    start=(i == 0), stop=(i == IC - 1),
                                )
                            y_sb = ypool.tile([P, 512], BF16, name="y_sb",
                                              tag="y")
                            nc.vector.tensor_tensor(
                                out=y_sb[:], in0=ps[:],
                                in1=b_sb[:, 512 * oh:512 * (oh + 1)],
                                op=ALU.add,
                            )
                            nc.sync.dma_start(
                                out=y[P * k:P * (k + 1),
                                      512 * oh:512 * (oh + 1)],
                                in_=y_sb[:],
                            )

                sim_phase(0)
                with tc.tile_wait_until(0.030):
                    ld_vfx(4, nc.sync)
                    ld_vfx(6, nc.sync)
                    ld_vfx(1, nc.scalar)
                    ld_vfx(3, nc.scalar)
                sim_phase(1)
                with tc.tile_wait_until(0.033):
                    ld_wo(0, nc.sync)
                    ld_wo(2, nc.sync)
                    for i in range(1, IC, 2):
                        nc.scalar.dma_start(out=XTb[i][:],
                                            in_=xT[P * i:P * (i + 1),
                                                   1024:2048])
                qmt1_phase()
                with tc.tile_wait_until(0.037):
                    ld_wo(4, nc.sync)
                    ld_wo(6, nc.sync)
                    ld_vfx(5, nc.scalar)
                    ld_vfx(7, nc.scalar)
                    ld_wo(1, nc.scalar)
                    ld_wo(3, nc.scalar)
                pv_phase(0)
                with tc.tile_wait_until(0.040):
                    nc.scalar.dma_start(out=b_sb[:], in_=bb[:])
                    ld_wo(5, nc.scalar)
                    ld_wo(7, nc.scalar)
                sim_phase(2)
                with tc.tile_wait_until(0.044):
                    ld_vfx(8, nc.sync)
                    ld_vfx(10, nc.sync)
                    ld_vfx(9, nc.scalar)
                    ld_vfx(11, nc.scalar)
                proj_phase(0)
                pv_phase(1)
                sim_phase(3)
                with tc.tile_wait_until(0.048):
                    ld_vfx(12, nc.sync)
                    ld_vfx(14, nc.sync)
                    ld_vfx(13, nc.scalar)
                    ld_vfx(15, nc.scalar)
                proj_phase(1)
                pv_phase(2)
                proj_phase(2)
                pv_phase(3)
                proj_phase(3)

    nc.compile()
    return nc


def _prep_inputs(x, w_qkv, w_out, b_out):
    import ml_dtypes
    BF = ml_dtypes.bfloat16
    x = np.asarray(x, dtype=np.float32)
    w_qkv = np.asarray(w_qkv, dtype=np.float32)
    w_out = np.asarray(w_out, dtype=np.float32)
    b_out = np.asarray(b_out, dtype=np.float32)

    wq = w_qkv[:, 0 * NI:1 * NI]
    wk = w_qkv[:, 1 * NI:2 * NI]
    wv = w_qkv[:, 2 * NI:3 * NI]
    mq = np.ascontiguousarray((wq @ wk.T).astype(BF))
    wvo = np.ascontiguousarray((wv @ w_out).astype(BF))
    b_bcast = np.ascontiguousarray(np.broadcast_to(b_out[None, :], (P, NO)))

    xbf = [x[b].astype(BF) for b in range(B)]
    xTs = [np.ascontiguousarray(xb.T) for xb in xbf]

    in_maps = []
    for c in range(NCORES):
        b, h = c // 2, c % 2
        subs = [2 * k + h for k in range(NSUB)]
        xQc = np.ascontiguousarray(np.concatenate(
            [xTs[b][:, P * s:P * (s + 1)] for s in subs], axis=1
        ))
        m = np.empty((NSUB, P, 512), dtype=BF)
        cpos = np.arange(512)[None, :]
        prow = np.arange(P)[:, None]
        for k in range(NSUB):
            if k % 2 == 0:
                # 256-wide diagonal chunk at key offset 256k
                off = P * subs[k] - 256 * k
                row = np.where(cpos <= off + prow, 0.0, NEG)
                row[:, 256:] = NEG
                m[k] = row
            else:
                off = P * subs[k] - 512 * (CC[k] - 1)
                m[k] = np.where(cpos <= off + prow, 0.0, NEG)
        in_maps.append({
            "xT": xTs[b], "xR": xbf[b],
            "mq": mq, "xq": xQc, "wvo": wvo,
            "masks": m, "bb": b_bcast,
        })
    return in_maps


def _run(x, w_qkv, w_out, b_out, trace=False, **kw):
    if "nc" not in _CACHED:
        _CACHED["nc"] = _build()
    nc = _CACHED["nc"]
    in_maps = _prep_inputs(x, w_qkv, w_out, b_out)
    res = run_bass_kernel_spmd(nc, in_maps, list(range(NCORES)), trace=trace, **kw)
    out = np.empty((B, S, NO), dtype=np.float32)
    for c in range(NCORES):
        b, h = c // 2, c % 2
        yc = np.asarray(res.results[c]["y"], dtype=np.float32)
        for k in range(NSUB):
            s = 2 * k + h
            out[b, P * s:P * (s + 1), :] = yc[P * k:P * (k + 1), :]
    return out, res


def kernel(x, w_qkv, w_out, b_out):
    out, _ = _run(x, w_qkv, w_out, b_out, trace=False)
    return out


# revision 15
# speedup vs baseline: 1.1794x; 1.1794x over previous
"""Causal single-head attention block for Trainium2, SPMD across 8 NeuronCores.

Problem (hardcoded):
    x:     [4, 2048, 1024] f32
    w_qkv: [1024, 3072]    f32   (q | k | v column blocks)
    w_out: [1024, 1024]    f32
    b_out: [1024]          f32
    y = softmax(causal(q @ k.T / 32)) @ v @ w_out + b_out     -> [4, 2048, 1024]

Algebraic folding (host-side, fp32):
    sim  = (x wq)(x wk)^T = x (wq wk^T) x^T          -> Mq  = wq @ wk.T
    out  = attn (x wv) wo = attn x (wv wo)           -> Wvo = wv @ w_out
so the device kernel never materializes Q/K/V: it computes
    QM^T = Mq^T x_q^T   (local queries only)
    sim  = QM x^T       (x^T SBUF-resident)
    attnX = softmax(causal(sim)) @ x                 (x rows SBUF-resident)
    y    = attnX @ Wvo + b

Sharding: 2 cores per batch element. Within a batch, the 16 query subtiles of
128 rows are dealt round-robin to the core pair (core parity h gets subtiles
s = 2k + h, k = 0..7) so both cores see the identical causal work profile
(512-key chunk counts [1,1,2,2,3,3,4,4]) and a single SPMD program serves all
8 cores; per-core behavior differs only through input data.

Schedule notes (v4, driven by ntff traces of earlier revisions):
  * The PE runs 512-free bf16 matmuls at a 216ns cadence once warm; wins are
    at the edges. HAM grants ~133.1us of full-rate PE per core then clamps
    to half rate, and any PE idle >~2us costs an extra ~3.4us half-rate
    quantum on resume, so the schedule avoids both long stalls and overrun.
  * Phase 0 accumulates all 8 QM^T d-chunks in the 8 PSUM banks at once
    (consumption ~1.7us per 512KB feed step vs ~1.4us arrival), with an
    8-matmul warm-up bridging the trigger preamble to first-chunk arrival.
    The PSUM->SBUF casts alternate vector/scalar, and the first sim
    subtile's contraction is split i=0..3 / i=4..7 so its first half only
    waits for half the casts.
  * DMA trigger instructions cost ~0.6-0.9us on the issuing engine and the
    DGE queue blocks the engine when its semaphore rotation (~4 deep) is
    exhausted, so: sync owns the phase-0-critical stream plus the early
    tail, scalar owns a 14-trigger static prefix and issues the rest in
    small stages between compute phases (its engine is otherwise idle
    there), and all 16 x row tiles are pinned in SBUF (one read each)
    instead of re-streaming rows per PV group (saves 3MB of wire).
  * PV skips the two fully-masked 128-key blocks of each even subtile
    (128-wide matmuls on the odd-subtile half only): -3.4us of PE.
  * proj groups 2/3 draw their y PSUM from the then-idle accp pool so the
    tensor engine never waits on a PSUM WAR against the immediately
    preceding PV accumulation.

All matmul operands are bf16 (PSUM accumulation in fp32; softmax statistics
in fp32): the elementwise rounding step is 4x fp32r's, far inside the
tolerance, and bf16 enables fast weight load + halves DMA/DVE traffic.
"""

import numpy as np

import concourse.mybir as mybir
import concourse.tile as tile
from concourse import bacc
from concourse.bass_utils import run_bass_kernel_spmd

FP32 = mybir.dt.float32
BF16 = mybir.dt.bfloat16
AF = mybir.ActivationFunctionType
ALU = mybir.AluOpType

B, S, D, NI, NO = 4, 2048, 1024, 1024, 1024
NCORES = 8
P = 128
DC = D // P    # 8 contraction chunks for the projections
IC = NI // P   # 8 inner-dim chunks
NSUB = 8       # local 128-row query subtiles per core
CC = [k // 2 + 1 for k in range(NSUB)]  # 512-key chunks per local subtile
SCALE = float(NI) ** -0.5
NEG = -1.0e9
NWARM = 8

_CACHED = {}


def _build():
    nc = bacc.Bacc(None, target_bir_lowering=False, debug=False, num_devices=NCORES)

    xT = nc.dram_tensor("xT", [D, S], BF16, kind="ExternalInput").ap()
    mq_d = nc.dram_tensor("mq", [D, D], BF16, kind="ExternalInput").ap()
    # local query columns of x^T, packed [low 512 | high 512]
    xq_d = nc.dram_tensor("xq", [D, NSUB * P], BF16, kind="ExternalInput").ap()
    xR = nc.dram_tensor("xR", [S, D], BF16, kind="ExternalInput").ap()
    wvo_d = nc.dram_tensor("wvo", [NI, NO], BF16, kind="ExternalInput").ap()
    masks = nc.dram_tensor("masks", [NSUB, P, 512], BF16, kind="ExternalInput").ap()
    bb = nc.dram_tensor("bb", [P, NO], FP32, kind="ExternalInput").ap()
    y = nc.dram_tensor("y", [NSUB * P, NO], BF16, kind="ExternalOutput").ap()

    with tile.TileContext(nc) as tc:
        with (
            tc.tile_pool(name="const", bufs=1) as constp,
            tc.tile_pool(name="xtpool", bufs=2 * IC) as xtp,
            tc.tile_pool(name="qtpool0", bufs=IC) as qtp0,
            tc.tile_pool(name="qtpool1", bufs=IC) as qtp1,
            tc.tile_pool(name="wpool", bufs=DC) as wp,
            tc.tile_pool(name="xqpool", bufs=2 * DC) as xqp,
            tc.tile_pool(name="vfixp", bufs=16) as vfixp,
            tc.tile_pool(name="wopool", bufs=DC) as wop,
        ):
            # two separate tile arrays per key-half: a write to the high half
            # must not create a false whole-tile WAR hazard against sim reads
            # of the low half (tile dependencies are tracked per-tile)
            XTa = [xtp.tile([P, S // 2], BF16, name=f"xta{i}", tag="xt")
                   for i in range(IC)]
            XTb = [xtp.tile([P, S // 2], BF16, name=f"xtb{i}", tag="xt")
                   for i in range(IC)]
            # per-qh-half QM^T tiles: keeps group 0's sim free of any
            # dependency on the second half's PSUM->SBUF copies
            QMT = [
                [qtp0.tile([P, 512], BF16, name=f"qt0_{i}", tag="qt0")
                 for i in range(IC)],
                [qtp1.tile([P, 512], BF16, name=f"qt1_{i}", tag="qt1")
                 for i in range(IC)],
            ]
            mq = [wp.tile([P, D], BF16, name=f"mq{d}", tag="w")
                  for d in range(DC)]
            xqlo = [xqp.tile([P, 512], BF16, name=f"xql{d}", tag="xq")
                    for d in range(DC)]
            xqhi = [xqp.tile([P, 512], BF16, name=f"xqh{d}", tag="xq")
                    for d in range(DC)]
            mask_sb = constp.tile([P, NSUB, 512], BF16, name="mask_sb", tag="mask")
            vfix = [vfixp.tile([P, NI], BF16, name=f"vfix{t}", tag="vfix")
                    for t in range(16)]
            wo = [wop.tile([P, NO], BF16, name=f"wo{d}", tag="wo")
                  for d in range(DC)]
            b_sb = constp.tile([P, NO], FP32, name="b_sb", tag="b")

            def ld_xq(d, half, eng):
                t = xqlo[d] if half == 0 else xqhi[d]
                eng.dma_start(out=t[:],
                              in_=xq_d[P * d:P * (d + 1),
                                       512 * half:512 * (half + 1)])

            def ld_mask(k, eng):
                eng.dma_start(out=mask_sb[:, k, :], in_=masks[k])

            def ld_vfx(t, eng):
                eng.dma_start(out=vfix[t][:], in_=xR[P * t:P * (t + 1), :])

            def ld_wo(d, eng):
                eng.dma_start(out=wo[d][:], in_=wvo_d[P * d:P * (d + 1), :])

            # ---- phase-0-critical feed: mq + xq-low interleaved so step d's
            # pair lands on opposite queues simultaneously
            for d in range(DC):
                if d % 2 == 0:
                    nc.sync.dma_start(out=mq[d][:], in_=mq_d[P * d:P * (d + 1), :])
                    ld_xq(d, 0, nc.scalar)
                else:
                    nc.scalar.dma_start(out=mq[d][:], in_=mq_d[P * d:P * (d + 1), :])
                    ld_xq(d, 0, nc.sync)
            for i in range(IC):
                eng = nc.sync if i % 2 == 0 else nc.scalar
                eng.dma_start(out=XTa[i][:], in_=xT[P * i:P * (i + 1), 0:1024])
            ld_mask(0, nc.sync)
            ld_mask(1, nc.scalar)
            ld_mask(2, nc.sync)
            ld_mask(3, nc.scalar)
            ld_mask(5, nc.scalar)
            ld_mask(7, nc.scalar)
            # scalar's static prefix stops at 16 triggers so its engine is
            # free for the phase-0 casts by ~24us. Everything else rides the
            # sync queue's tail in need order (the queue's FIFO transfer
            # order doubles as the HBM priority order): sim-group-2/3 keys,
            # qmt1's xq-high, masks 4/6, the first PV x rows, Wvo, bias.
            for i in range(IC):
                nc.sync.dma_start(out=XTb[i][:],
                                  in_=xT[P * i:P * (i + 1), 1024:2048])
            for d in range(DC):
                ld_xq(d, 1, nc.sync)
            ld_mask(4, nc.sync)
            ld_mask(6, nc.sync)
            for t in range(4):
                ld_vfx(t, nc.sync)

            # ---- Phase 0: QM^T = Mq^T @ xQ for queries 0:512 (groups 0/1),
            # all 8 output chunks accumulating in the 8 PSUM banks at once so
            # the PE tracks the dual-queue feed with no round structure.
            with tc.tile_pool(name="qacc", bufs=8, space="PSUM") as qacc:
                # HAM warm-up: bridges the ~3us between the trigger preamble
                # and the first mq/xq chunk landing, and starts the clock
                # ramp-up early. Sized to end roughly at first-chunk arrival.
                warm = constp.tile([P, 512], BF16, name="warm", tag="warm")
                nc.vector.memset(warm[:], 0.0)
                wps = qacc.tile([P, 512], FP32, name="warmps", tag="qa")
                for w in range(NWARM):
                    nc.tensor.matmul(
                        wps[:], warm[:, 0:P], warm[:],
                        start=(w == 0), stop=(w == NWARM - 1),
                    )
                pss = [
                    qacc.tile([P, 512], FP32, name=f"ps_qt0_{j}", tag="qa")
                    for j in range(IC)
                ]
                for d in range(DC):
                    for j in range(IC):
                        nc.tensor.matmul(
                            pss[j][:],
                            mq[d][:, P * j:P * (j + 1)],
                            xqlo[d][:],
                            start=(d == 0), stop=(d == DC - 1),
                        )
                # casts alternate vector/scalar; bank j's cast chains off its
                # own d=7 matmul (banks complete 216ns apart), so each
                # engine's 4-cast chain ends ~1.7us after the last matmul
                for j in range(IC):
                    if j % 2 == 0:
                        nc.vector.tensor_copy(QMT[0][j][:], pss[j][:])
                    else:
                        nc.scalar.activation(QMT[0][j][:], pss[j][:], AF.Copy)

            # remainder of the sync tail: projection weights + bias
            for d in range(DC):
                ld_wo(d, nc.sync)
            nc.sync.dma_start(out=b_sb[:], in_=bb[:])

            with (
                tc.tile_pool(name="accp", bufs=4, space="PSUM") as accp,
                tc.tile_pool(name="opp", bufs=4, space="PSUM") as opp,
                tc.tile_pool(name="ppool", bufs=4) as ppool,
                tc.tile_pool(name="ptpool", bufs=2) as ptpool,
                tc.tile_pool(name="otpool", bufs=2 * IC) as otpool,
                tc.tile_pool(name="ypool", bufs=4) as ypool,
                tc.tile_pool(name="stp", bufs=24) as stp,
            ):
                PS = {}
                PTB = {}
                OT = {}

                def qmt1_phase():
                    # second query half (groups 2/3): d-inner accumulation
                    # through the accp ping-pong, overlapped with the
                    # group-0/1 attention pipeline
                    for i in range(IC):
                        ps = accp.tile([P, 512], FP32, name="ps_qt1",
                                       tag="acc")
                        for d in range(DC):
                            nc.tensor.matmul(
                                ps[:], mq[d][:, P * i:P * (i + 1)],
                                xqhi[d][:],
                                start=(d == 0), stop=(d == DC - 1),
                            )
                        if i % 2 == 0:
                            nc.vector.tensor_copy(QMT[1][i][:], ps[:])
                        else:
                            nc.scalar.activation(QMT[1][i][:], ps[:], AF.Copy)

                def sim_phase(g):
                    L = g + 1
                    nt = 4 * L
                    ptb = ptpool.tile([P, 16, 256], BF16, name=f"ptb{g}",
                                      tag="ptb")
                    for k in (2 * g, 2 * g + 1):
                        # exact causal need: subtile k covers 256*(k+1)
                        # keys. Even k: k/2 full 512 chunks + a 256-wide
                        # diagonal chunk; odd k: (k+1)/2 chunks with the
                        # diagonal mask in the last 512 chunk.
                        even = (k % 2 == 0)
                        ndiag = 256 if even else 512
                        doff = 256 * k if even else 512 * (L - 1)
                        nfull = k // 2 if even else L - 1
                        p_t = ppool.tile([P, 4 * 512], BF16, name=f"p{k}",
                                         tag="p")
                        sums = stp.tile([P, 4], FP32, name=f"sums{k}",
                                        tag="sums")
                        # diagonal chunk first: its mask+exp chain overlaps
                        # the remaining chunks' matmuls
                        for ci, kc in enumerate([-1] + list(range(nfull))):
                            ps = accp.tile([P, 512], FP32, name="ps_sim",
                                           tag="acc")
                            if kc < 0:
                                off, w = doff, ndiag
                            else:
                                off, w = 512 * kc, 512
                            psv = ps[:, 0:w]
                            # group 0's first chunks split the contraction
                            # in half so the first segment only waits on the
                            # first half of the phase-0 casts
                            segs = ((0, 4), (4, 8)) if g == 0 else ((0, 8),)
                            for i0, i1 in segs:
                                for i in range(i0, i1):
                                    xth = XTa[i] if off < 1024 else XTb[i]
                                    nc.tensor.matmul(
                                        psv,
                                        QMT[k // 4][i][:, P * (k % 4):
                                                       P * (k % 4) + P],
                                        xth[:, off % 1024:off % 1024 + w],
                                        start=(i == 0), stop=(i == IC - 1),
                                    )
                            if kc < 0:
                                nc.vector.tensor_tensor(
                                    out=psv, in0=psv,
                                    in1=mask_sb[:, k, 0:w],
                                    op=ALU.add,
                                )
                            nc.scalar.activation(
                                p_t[:, off:off + w], psv, AF.Exp,
                                scale=SCALE, accum_out=sums[:, ci:ci + 1],
                            )
                        ssum = stp.tile([P, 1], FP32, name=f"ssum{k}", tag="ss")
                        nc.vector.tensor_reduce(
                            ssum[:], sums[:, :nfull + 1],
                            axis=mybir.AxisListType.X,
                            op=ALU.add,
                        )
                        rsum = stp.tile([P, 1], FP32, name=f"rsum{k}", tag="rs")
                        nc.vector.reciprocal(rsum[:], ssum[:])
                        nc.vector.tensor_scalar_mul(
                            p_t[:, :256 * (k + 1)], p_t[:, :256 * (k + 1)],
                            rsum[:]
                        )
                        PS[k] = p_t
                        # one XBAR DMA per subtile transposes the whole
                        # normalized-probability slab into blocked
                        # [key, t, q] layout; issued here so it overlaps
                        # the previous group's PV/projection work. The
                        # even subtile's last two key blocks are fully
                        # masked and simply never written nor read.
                        half = slice(0, P) if even else slice(P, 256)
                        nc.sync.dma_start(
                            out=ptb[:, 0:2 * (k + 1), half],
                            in_=p_t[:, 0:256 * (k + 1)],
                            transpose=True,
                        )
                    PTB[g] = ptb

                def pv_phase(g):
                    L = g + 1
                    ops = [
                        opp.tile([P, 512], FP32, name=f"op{g}_{j}", tag="op")
                        for j in range(4)
                    ]
                    nt = 4 * L
                    ptb = PTB[g]
                    # fetch the x rows this group newly needs (each row tile
                    # is loaded exactly once and stays pinned for later
                    # groups); issued here so the transfers ride the sync
                    # queue one group ahead of their first use
                    if g >= 1:
                        for t in range(4 * g, 4 * g + 4):
                            ld_vfx(t, nc.sync)
                    for t in range(nt):
                        v_t = vfix[t]
                        # the even subtile's last two key blocks are fully
                        # masked: matmul only the odd-subtile query half
                        narrow = t >= nt - 2
                        for m in range(IC):
                            if narrow:
                                out = ops[m // 2][:, 256 * (m % 2) + P:
                                                  256 * (m % 2) + 256]
                                rhs = ptb[:, t, P:256]
                            else:
                                out = ops[m // 2][:, 256 * (m % 2):
                                                  256 * (m % 2) + 256]
                                rhs = ptb[:, t, :]
                            # one accumulation group per PSUM bank: start
                            # only on the bank's first matmul (whole-bank
                            # pending-zero makes the sibling column-half's
                            # first write an overwrite), stop on its last
                            nc.tensor.matmul(
                                out,
                                v_t[:, P * m:P * (m + 1)],
                                rhs,
                                start=(t == 0 and m % 2 == 0),
                                stop=(t == nt - 1 and m % 2 == 1),
                            )

                    oT = []
                    for m in range(IC):
                        ot = otpool.tile([P, 256], BF16, name=f"ot{g}_{m}",
                                         tag="ot")
                        nc.vector.tensor_copy(
                            ot[:],
                            ops[m // 2][:, 256 * (m % 2):256 * (m % 2) + 256]
                        )
                        oT.append(ot)
                    OT[g] = oT

                def proj_phase(g):
                    # deferred past the next group's sim matmuls so the
                    # serialized attnX^T PSUM->SBUF casts don't stall the
                    # tensor engine between PV and projection
                    k0, k1 = 2 * g, 2 * g + 1
                    oT = OT[g]
                    # groups 0/1 draw y PSUM from opp (accp is busy with the
                    # interleaved sim); groups 2/3 draw from the then-idle
                    # accp so the first y matmul never waits a WAR against
                    # the PV accumulation that just stopped
                    ypool_ps = opp if g < 2 else accp
                    ytag = "op" if g < 2 else "acc"
                    for col, k in enumerate((k0, k1)):
                        for oh in range(2):
                            ps = ypool_ps.tile([P, 512], FP32, name="ps_y",
                                               tag=ytag)
                            for i in range(IC):
                                nc.tensor.matmul(
                                    ps[:],
                                    oT[i][:, P * col:P * (col + 1)],
                                    wo[i][:, 512 * oh:512 * (oh + 1)],
                                    start=(i == 0), stop=(i == IC - 1),
                                )
                            y_sb = ypool.tile([P, 512], BF16, name="y_sb",
                                              tag="y")
                            nc.vector.tensor_tensor(
                                out=y_sb[:], in0=ps[:],
                                in1=b_sb[:, 512 * oh:512 * (oh + 1)],
                                op=ALU.add,
                            )
                            nc.sync.dma_start(
                                out=y[P * k:P * (k + 1),
                                      512 * oh:512 * (oh + 1)],
                                in_=y_sb[:],
                            )

                sim_phase(0)
                sim_phase(1)
                qmt1_phase()
                pv_phase(0)
                sim_phase(2)
                proj_phase(0)
                pv_phase(1)
                sim_phase(3)
                proj_phase(1)
                pv_phase(2)
                proj_phase(2)
                pv_phase(3)
                proj_phase(3)

    nc.compile()
    return nc


def _prep_inputs(x, w_qkv, w_out, b_out):
    import ml_dtypes
    BF = ml_dtypes.bfloat16
    x = np.asarray(x, dtype=np.float32)
    w_qkv = np.asarray(w_qkv, dtype=np.float32)
    w_out = np.asarray(w_out, dtype=np.float32)
    b_out = np.asarray(b_out, dtype=np.float32)

    wq = w_qkv[:, 0 * NI:1 * NI]
    wk = w_qkv[:, 1 * NI:2 * NI]
    wv = w_qkv[:, 2 * NI:3 * NI]
    mq = np.ascontiguousarray((wq @ wk.T).astype(BF))
    wvo = np.ascontiguousarray((wv @ w_out).astype(BF))
    b_bcast = np.ascontiguousarray(np.broadcast_to(b_out[None, :], (P, NO)))

    xbf = [x[b].astype(BF) for b in range(B)]
    xTs = [np.ascontiguousarray(xb.T) for xb in xbf]

    in_maps = []
    for c in range(NCORES):
        b, h = c // 2, c % 2
        subs = [2 * k + h for k in range(NSUB)]
        xQc = np.ascontiguousarray(np.concatenate(
            [xTs[b][:, P * s:P * (s + 1)] for s in subs], axis=1
        ))
        m = np.empty((NSUB, P, 512), dtype=BF)
        cpos = np.arange(512)[None, :]
        prow = np.arange(P)[:, None]
        for k in range(NSUB):
            if k % 2 == 0:
                # 256-wide diagonal chunk at key offset 256k
                off = P * subs[k] - 256 * k
                row = np.where(cpos <= off + prow, 0.0, NEG)
                row[:, 256:] = NEG
                m[k] = row
            else:
                off = P * subs[k] - 512 * (CC[k] - 1)
                m[k] = np.where(cpos <= off + prow, 0.0, NEG)
        in_maps.append({
            "xT": xTs[b], "xR": xbf[b],
            "mq": mq, "xq": xQc, "wvo": wvo,
            "masks": m, "bb": b_bcast,
        })
    return in_maps


def _run(x, w_qkv, w_out, b_out, trace=False, **kw):
    if "nc" not in _CACHED:
        _CACHED["nc"] = _build()
    nc = _CACHED["nc"]
    in_maps = _prep_inputs(x, w_qkv, w_out, b_out)
    res = run_bass_kernel_spmd(nc, in_maps, list(range(NCORES)), trace=trace, **kw)
    out = np.empty((B, S, NO), dtype=np.float32)
    for c in range(NCORES):
        b, h = c // 2, c % 2
        yc = np.asarray(res.results[c]["y"], dtype=np.float32)
        for k in range(NSUB):
            s = 2 * k + h
            out[b, P * s:P * (s + 1), :] = yc[P * k:P * (k + 1), :]
    return out, res


def kernel(x, w_qkv, w_out, b_out):
    out, _ = _run(x, w_qkv, w_out, b_out, trace=False)
    return out
